# revision 32
# baseline (speedup 1.0000x reference)
"""
Trainium2 Bass kernel for nn_Attention_29265907155069.

Reference computation (B=4, N=2048, C=768, H=12, D=64):
    qkv = x @ qkv_w.T -> split to q,k,v per head
    attn = softmax(q @ k.T * D + mask * -1e6)
    out  = (attn @ v) re-concat -> @ proj_w.T + proj_b
Scores are ~N(0, 512^2); any mask bias larger than the score span
(~4000) reproduces the reference exactly (here -8192, exact in fp8e5m2).

Sharding: 8 cores = (batch b in 0..3) x (head-group hg in 0..1, 6 heads each).
Each core computes its 6 heads' attention for its batch and a row-sharded
partial of the output projection; host sums the two head-group partials.

Per-core device pipeline:
  1. QKV: Q^T,K^T [d, n] and V [k, d] via PE matmuls (float32r). The D=64
     softmax scale is folded into Q on the host (Q-weights * 64).
  2. Scores S = 64*q@k.T per q-tile into PSUM (f32r); an fp8e5m2 identity
     matmul accumulates -8192 * mask onto the same PSUM tile.
  3. Per 1024-col half: DVE reduce_max(negate) -> -m_half, ACT exp with
     bias=-m_half -> fp16 (early exp: frees PSUM fast, keeps PE fed).
  4. fs = exp(m_half - m) on ACT; DVE rescales each half (fp16 2x).
  5. DMA xbar transpose P -> P^T per half, all on the sync HWDGE queue
     (concurrent transposes on both HWDGE queues corrupt data); every
     other phase-2 DMA is routed to the scalar HWDGE / gpsimd SWDGE
     queues to keep the sync queue transpose-only.
  4b. The exp's accum_out gives row-sums l_half for free; the rescale
     factor becomes fs_half/l (1/l folded in, all [128,1] ops) so P is
     pre-normalized before the transpose.
  6. PV: O^T[64, q] = V.T @ P^T accumulated over k tiles; PV matmuls are
     interleaved between score groups to fill PE stalls (p-state ramp).
  7. O^T copied to fp16 Ocat on ACT (already normalized).
  8. proj: Y[q, 768] = O^T.T @ projT (fp16) -> fp32 SBUF -> DMA (gpsimd).
"""

import os
import sys

import numpy as np

for _p in ("/opt/trn_rl_repo", "/root/.axon_site/_ro/trn_rl_repo"):
    if os.path.isdir(_p) and _p not in sys.path:
        sys.path.insert(0, _p)

import ml_dtypes  # noqa: E402

import concourse.mybir as mybir  # noqa: E402
from concourse import bacc  # noqa: E402
from concourse.bass_utils import run_bass_kernel_spmd  # noqa: E402
from concourse.tile import TileContext  # noqa: E402

B, N, C, H = 4, 2048, 768, 12
D = C // H          # 64
HG = 2              # head groups (cores per batch)
HPC = H // HG       # heads per core = 6
CIN_T = C // 128    # 6 cin tiles
QT_TILES = 3        # 384 rows of Q^T (6 heads x 64) = 3 x 128
KT_TILES = N // 128  # 16
NCORES = 8
MASK_BIAS = -8192.0

F32 = mybir.dt.float32
F32R = mybir.dt.float32r
F16 = mybir.dt.float16
F8E5 = mybir.dt.float8e5

_CACHE = {}


def _V(k):
    return os.environ.get(k)


def _build_program():
    nc = bacc.Bacc(
        "TRN2",
        target_bir_lowering=False,
        debug=False,
        enable_asserts=False,
        num_devices=NCORES,
    )
    xT = nc.dram_tensor("xT", [C, N], F32R, kind="ExternalInput").ap()
    qkvT = nc.dram_tensor("qkvT", [C, 3 * HPC * D], F32R, kind="ExternalInput").ap()
    maskdr = nc.dram_tensor("maskdr", [N // 2, 2 * N], F8E5, kind="ExternalInput").ap()
    identdr = nc.dram_tensor("identdr", [64, 256], F8E5, kind="ExternalInput").ap()
    identdr2 = nc.dram_tensor("identdr2", [128, 256], F8E5, kind="ExternalInput").ap()
    projT = nc.dram_tensor("projT", [HPC * D, C], F16, kind="ExternalInput").ap()
    out = nc.dram_tensor("out", [N, C], F32, kind="ExternalOutput").ap()

    AL = mybir.AluOpType
    EXP = mybir.ActivationFunctionType.Exp
    LN = mybir.ActivationFunctionType.Ln

    with TileContext(nc) as tc:
        with tc.tile_pool(name="pers", bufs=1) as pers:
            QTs = [
                pers.tile([128, N], F32R, tag=f"qt{t}", name=f"qt{t}")
                for t in range(QT_TILES)
            ]
            KTs = [
                pers.tile([128, N], F32R, tag=f"kt{t}", name=f"kt{t}")
                for t in range(QT_TILES)
            ]
            Vaug = pers.tile([128, HPC * KT_TILES, D], F16, tag="vaug")
            Ocat = [
                pers.tile([128, N], F16, tag=f"oc{t}", name=f"oc{t}")
                for t in range(QT_TILES)
            ]
            PW = [
                pers.tile([128, C], F16, tag=f"pw{t}", name=f"pw{t}")
                for t in range(QT_TILES)
            ]
            if _V("KV_DR"):
                idr = pers.tile([128, 2, 128], F8E5, tag="idr")
                nc.sync.dma_start(
                    idr[:, :, :],
                    identdr2.rearrange("p (i q) -> p i q", i=2),
                )
            else:
                idr = pers.tile([128, 128], F8E5, tag="idr")
                nc.sync.dma_start(
                    idr[:, :], identdr.rearrange("p (i q) -> (p i) q", i=2)
                )
            for t in range(QT_TILES):
                nc.scalar.dma_start(PW[t][:, :], projT[t * 128 : (t + 1) * 128, :])

            # ================= Phase 1: QKV projection =================
            with (
                tc.tile_pool(name="ph1", bufs=1) as p1,
                tc.tile_pool(name="ph1p", bufs=4, space="PSUM") as p1p,
            ):
                xts = [
                    p1.tile([128, N], F32R, tag=f"x{ci}", name=f"x{ci}")
                    for ci in range(CIN_T)
                ]
                wts = [
                    p1.tile([128, 3 * HPC * D], F32R, tag=f"w{ci}", name=f"w{ci}")
                    for ci in range(CIN_T)
                ]
                for ci in range(CIN_T):
                    nc.sync.dma_start(xts[ci][:, :], xT[ci * 128 : (ci + 1) * 128, :])
                    nc.scalar.dma_start(
                        wts[ci][:, :], qkvT[ci * 128 : (ci + 1) * 128, :]
                    )

                for which, dst in ((0, QTs), (1, KTs)):
                    off = which * HPC * D
                    for t in range(QT_TILES):
                        for qc in range(4):
                            ps = p1p.tile([128, 512], F32, tag="p1ps", name="ps")
                            for ci in range(CIN_T):
                                nc.tensor.matmul(
                                    ps[:, :],
                                    wts[ci][:, off + t * 128 : off + (t + 1) * 128],
                                    xts[ci][:, qc * 512 : (qc + 1) * 512],
                                    start=(ci == 0),
                                    stop=(ci == CIN_T - 1),
                                )
                            nc.vector.tensor_scalar(
                                dst[t][:, qc * 512 : (qc + 1) * 512],
                                ps[:, :], 0.0, None, op0=AL.add,
                            )

                voff = 2 * HPC * D
                for kt in range(KT_TILES):
                    ps = p1p.tile([128, HPC * D], F32, tag="p1ps", name="ps")
                    for ci in range(CIN_T):
                        nc.tensor.matmul(
                            ps[:, :],
                            xts[ci][:, kt * 128 : (kt + 1) * 128],
                            wts[ci][:, voff : voff + HPC * D],
                            start=(ci == 0),
                            stop=(ci == CIN_T - 1),
                        )
                    nc.scalar.copy(
                        Vaug[:, kt :: KT_TILES, :],
                        ps[:, :].rearrange("p (h d) -> p h d", h=HPC),
                    )

            # ================= Phase 2: attention =================
            with (
                tc.tile_pool(name="mk", bufs=2) as pmk,
                tc.tile_pool(name="work", bufs=2) as pw,
                tc.tile_pool(name="psS", bufs=3, space="PSUM") as psS,
                tc.tile_pool(name="psO", bufs=2, space="PSUM") as psO,
            ):
                pv_pending = []  # fine-grained PE filler closures

                def make_pv(hp, qc, PTs_blk):
                    """Two closure pairs per head: 8+8 PV matmuls, then post."""
                    parts = []
                    for a in range(2):
                        h = 2 * hp + a
                        ot = [None]

                        def chain_lo(a=a, h=h, PTs_blk=PTs_blk, ot=ot):
                            ot[0] = psO.tile([D, 512], F32, tag="ot", name="ot")
                            for kt in range(8):
                                nc.tensor.matmul(
                                    ot[0][:, :],
                                    Vaug[:, h * KT_TILES + kt, :],
                                    PTs_blk[a][0][:, kt, :],
                                    start=(kt == 0),
                                    stop=False,
                                )

                        def chain_hi(a=a, h=h, PTs_blk=PTs_blk, ot=ot,
                                     hp=hp, qc=qc):
                            for kt in range(8, KT_TILES):
                                nc.tensor.matmul(
                                    ot[0][:, :],
                                    Vaug[:, h * KT_TILES + kt, :],
                                    PTs_blk[a][1][:, kt - 8, :],
                                    start=False,
                                    stop=(kt == KT_TILES - 1),
                                )
                            nc.vector.tensor_scalar(
                                Ocat[hp][a * D : (a + 1) * D,
                                         qc * 512 : (qc + 1) * 512],
                                ot[0][:, :], 0.0, None, op0=AL.add,
                            )

                        parts.append(chain_lo)
                        parts.append(chain_hi)
                    return parts

                for qc in range(4):
                    mks = []
                    for j in range(4):
                        qt = qc * 4 + j
                        mk = pmk.tile([128, N], F8E5, tag=f"mk{j}", name=f"mk{j}")
                        nc.sync.dma_start(
                            mk[:, :],
                            maskdr[qt * 64 : (qt + 1) * 64, :].rearrange(
                                "p (i n) -> (p i) n", i=2
                            ),
                        )
                        mks.append(mk)

                    for hp in range(QT_TILES):
                        PTs = [
                            [
                                pw.tile(
                                    [128, 8, 512], F16, tag=f"pt{a}{hh}",
                                    name=f"PT{a}{hh}", bufs=2,
                                )
                                for hh in range(2)
                            ]
                            for a in range(2)
                        ]
                        for j in range(4):
                            qt = qc * 4 + j
                            for a in range(2):
                                mstat = pw.tile([128, 2], F32, tag=f"mstat{a}",
                                                name=f"mstat{a}", bufs=6)
                                negm = pw.tile([128, 1], F32, tag=f"negm{a}",
                                               name=f"negm{a}", bufs=6)
                                fs = pw.tile([128, 2], F32, tag=f"fs{a}",
                                             name=f"fs{a}", bufs=6)
                                lsum = pw.tile([128, 2], F32, tag=f"ls{a}",
                                               name=f"ls{a}", bufs=6)
                                lf = pw.tile([128, 2], F32, tag=f"lf{a}",
                                             name=f"lf{a}", bufs=6)
                                li = pw.tile([128, 1], F32, tag=f"li{a}",
                                             name=f"li{a}", bufs=6)
                                lr = pw.tile([128, 1], F32, tag=f"lr{a}",
                                             name=f"lr{a}", bufs=6)
                                fac = pw.tile([128, 2], F32, tag=f"fa{a}",
                                              name=f"fa{a}", bufs=6)
                                pnh = [
                                    pw.tile([128, 1024], F16, tag=f"pn{a}{hh}",
                                            name=f"pn{a}{hh}", bufs=2)
                                    for hh in range(2)
                                ]
                                for half in range(2):
                                    sp = psS.tile([128, 1024], F32, tag="sp",
                                                  name=f"sp{a}{half}")
                                    for c in range(2):
                                        kc = half * 1024 + c * 512
                                        nc.tensor.matmul(
                                            sp[:, c * 512 : (c + 1) * 512],
                                            QTs[hp][
                                                a * D : (a + 1) * D,
                                                qt * 128 : (qt + 1) * 128,
                                            ],
                                            KTs[hp][
                                                a * D : (a + 1) * D,
                                                kc : kc + 512,
                                            ],
                                            start=True,
                                            stop=False,
                                            tile_position=(a * D, 0),
                                        )
                                        if _V("KV_DR"):
                                            nc.tensor.matmul(
                                                sp[:, c * 512 : (c + 1) * 512],
                                                idr[:, :, :],
                                                mks[j][:, kc : kc + 512]
                                                .rearrange("p (o n) -> p o n", o=1)
                                                .broadcast_to([128, 2, 512]),
                                                start=False,
                                                stop=True,
                                                perf_mode=mybir.MatmulPerfMode.DoubleRow,
                                            )
                                        else:
                                            nc.tensor.matmul(
                                                sp[:, c * 512 : (c + 1) * 512],
                                                idr[:, :],
                                                mks[j][:, kc : kc + 512],
                                                start=False,
                                                stop=True,
                                            )
                                    nc.vector.tensor_reduce(
                                        mstat[:, half : half + 1],
                                        sp[:, :],
                                        axis=mybir.AxisListType.X,
                                        op=AL.max,
                                        negate=True,
                                    )
                                    if _V("KV_LSUMDVE"):
                                        nc.scalar.activation(
                                            pnh[half][:, :],
                                            sp[:, :],
                                            EXP,
                                            bias=mstat[:, half : half + 1],
                                            scale=1.0,
                                        )
                                        nc.vector.tensor_reduce(
                                            lsum[:, half : half + 1],
                                            pnh[half][:, :],
                                            axis=mybir.AxisListType.X,
                                            op=AL.add,
                                        )
                                    else:
                                        nc.scalar.activation(
                                            pnh[half][:, :],
                                            sp[:, :],
                                            EXP,
                                            bias=mstat[:, half : half + 1],
                                            scale=1.0,
                                            accum_out=lsum[:, half : half + 1],
                                        )
                                nc.vector.tensor_reduce(
                                    negm[:, :],
                                    mstat[:, 0:2],
                                    axis=mybir.AxisListType.X,
                                    op=AL.min,
                                )
                                nc.scalar.activation(
                                    fs[:, :],
                                    mstat[:, 0:2],
                                    EXP,
                                    bias=negm[:, 0:1],
                                    scale=-1.0,
                                )
                                # l = lsum0*fs0 + lsum1*fs1; factor = fs/l
                                nc.vector.tensor_tensor(
                                    lf[:, :], lsum[:, :], fs[:, :], op=AL.mult
                                )
                                nc.vector.tensor_reduce(
                                    li[:, :], lf[:, :],
                                    axis=mybir.AxisListType.X, op=AL.add,
                                )
                                nc.vector.reciprocal(lr[:, :], li[:, :])
                                nc.vector.tensor_scalar(
                                    fac[:, :], fs[:, :], lr[:, 0:1], None,
                                    op0=AL.mult,
                                )
                                for half in range(2):
                                    nc.vector.tensor_scalar(
                                        pnh[half][:, :],
                                        pnh[half][:, :],
                                        fac[:, half : half + 1],
                                        None,
                                        op0=AL.mult,
                                    )
                                    nc.sync.dma_start_transpose(
                                        PTs[a][half][:, :, j * 128 : (j + 1) * 128],
                                        pnh[half][:, :],
                                    )
                                # PE filler between head groups
                                if not _V("KV_NOILV") and pv_pending:
                                    pv_pending.pop(0)()
                        pv_pending.extend(make_pv(hp, qc, PTs))
                        if _V("KV_NOILV") or hp == QT_TILES - 1:
                            while pv_pending:
                                pv_pending.pop(0)()

                    # proj for this q-chunk
                    for j in range(4):
                        qt = qc * 4 + j
                        y0 = psO.tile([128, 512], F32, tag="ot", name="y0")
                        y1 = psO.tile([128, 256], F32, tag="ot", name="y1")
                        for ct in range(QT_TILES):
                            lt = Ocat[ct][:, qt * 128 : (qt + 1) * 128]
                            nc.tensor.matmul(
                                y0[:, :], lt, PW[ct][:, 0:512],
                                start=(ct == 0), stop=(ct == QT_TILES - 1),
                            )
                            nc.tensor.matmul(
                                y1[:, :], lt, PW[ct][:, 512:768],
                                start=(ct == 0), stop=(ct == QT_TILES - 1),
                            )
                        ysb = pw.tile([128, C], F32, tag="ysb", name="ysb")
                        nc.vector.tensor_scalar(
                            ysb[:, 0:512], y0[:, :], 0.0, None, op0=AL.add
                        )
                        if _V("KV_Y1DVE"):
                            nc.vector.tensor_scalar(
                                ysb[:, 512:768], y1[:, :], 0.0, None, op0=AL.add
                            )
                        else:
                            nc.scalar.copy(ysb[:, 512:768], y1[:, :])
                        eng = nc.sync if qt % 2 == 0 else nc.scalar
                        eng.dma_start(
                            out[qt * 128 : (qt + 1) * 128, :], ysb[:, :]
                        )
    nc.compile()
    return nc


def kernel(x, local_attn_mask, qkv_w, proj_w, proj_b):
    x = np.asarray(x, dtype=np.float32)
    mask = np.asarray(local_attn_mask)
    qkv_w = np.asarray(qkv_w, dtype=np.float32)
    proj_w = np.asarray(proj_w, dtype=np.float32)
    proj_b = np.asarray(proj_b, dtype=np.float32)

    maskdr = (
        (float(MASK_BIAS) * mask.astype(np.float32))
        .reshape(N // 128, 64, 2, N)
        .reshape(N // 2, 2 * N)
        .astype(ml_dtypes.float8_e5m2)
    )
    identdr = (
        np.eye(128, dtype=np.float32)
        .reshape(64, 2, 128)
        .reshape(64, 256)
        .astype(ml_dtypes.float8_e5m2)
    )
    id2 = np.zeros((128, 2, 128), dtype=np.float32)
    id2[:, 0, :] = np.eye(128, dtype=np.float32)
    identdr2 = id2.reshape(128, 256).astype(ml_dtypes.float8_e5m2)
    in_maps = []
    for c in range(NCORES):
        b, hg = c // HG, c % HG
        rq = slice(hg * HPC * D, (hg + 1) * HPC * D)
        rk = slice(C + hg * HPC * D, C + (hg + 1) * HPC * D)
        rv = slice(2 * C + hg * HPC * D, 2 * C + (hg + 1) * HPC * D)
        wsel = np.concatenate(
            [qkv_w[rq] * float(D), qkv_w[rk], qkv_w[rv]], axis=0
        )
        in_maps.append(
            {
                "xT": np.ascontiguousarray(x[b].T),
                "qkvT": np.ascontiguousarray(wsel.T),
                "maskdr": maskdr,
                "identdr": identdr,
                "identdr2": identdr2,
                "projT": np.ascontiguousarray(
                    proj_w[:, hg * HPC * D : (hg + 1) * HPC * D].T
                ).astype(np.float16),
            }
        )

    if "nc" not in _CACHE:
        _CACHE["nc"] = _build_program()
    res = run_bass_kernel_spmd(_CACHE["nc"], in_maps, core_ids=list(range(NCORES)))
    _CACHE["last_results"] = res
    outs = res.results
    y = np.empty((B, N, C), dtype=np.float32)
    for b in range(B):
        y[b] = outs[2 * b]["out"] + outs[2 * b + 1]["out"] + proj_b[None, :]
    return y


# revision 34
# speedup vs baseline: 1.1887x; 1.1887x over previous
"""
Trainium2 Bass kernel for nn_Attention_29265907155069.

Reference computation (B=4, N=2048, C=768, H=12, D=64):
    qkv = x @ qkv_w.T -> split to q,k,v per head
    attn = softmax(q @ k.T * D + mask * -1e6)
    out  = (attn @ v) re-concat -> @ proj_w.T + proj_b
Scores are ~N(0, 512^2); any mask bias larger than the score span
(~4000) reproduces the reference exactly (here -8192, exact in fp8e5m2).

Sharding: 8 cores = (batch b in 0..3) x (head-group hg in 0..1, 6 heads each).
Each core computes its 6 heads' attention for its batch and a row-sharded
partial of the output projection; host sums the two head-group partials.

Per-core device pipeline:
  1. QKV: Q^T,K^T [d, n] and V [k, d] via PE matmuls (float32r). The D=64
     softmax scale is folded into Q on the host (Q-weights * 64).
  2. Scores S = 64*q@k.T per q-tile into PSUM (f32r); an fp8e5m2 identity
     matmul accumulates -8192 * mask onto the same PSUM tile.
  3. Per 1024-col half: DVE reduce_max(negate) -> -m_half, ACT exp with
     bias=-m_half -> fp16 (early exp: frees PSUM fast, keeps PE fed).
  4. fs = exp(m_half - m) on ACT; DVE rescales each half (fp16 2x).
  5. DMA xbar transpose P -> P^T per half, all on the sync HWDGE queue
     (concurrent transposes on both HWDGE queues corrupt data); every
     other phase-2 DMA is routed to the scalar HWDGE / gpsimd SWDGE
     queues to keep the sync queue transpose-only.
  4b. The exp's accum_out gives row-sums l_half for free; the rescale
     factor becomes fs_half/l (1/l folded in, all [128,1] ops) so P is
     pre-normalized before the transpose.
  6. PV: O^T[64, q] = V.T @ P^T accumulated over k tiles; PV matmuls are
     interleaved between score groups to fill PE stalls (p-state ramp).
  7. O^T copied to fp16 Ocat on ACT (already normalized).
  8. proj: Y[q, 768] = O^T.T @ projT (fp16) -> fp32 SBUF -> DMA (gpsimd).
"""

import os
import sys

import numpy as np

for _p in ("/opt/trn_rl_repo", "/root/.axon_site/_ro/trn_rl_repo"):
    if os.path.isdir(_p) and _p not in sys.path:
        sys.path.insert(0, _p)

import ml_dtypes  # noqa: E402

import concourse.mybir as mybir  # noqa: E402
from concourse import bacc  # noqa: E402
from concourse.bass_utils import run_bass_kernel_spmd  # noqa: E402
from concourse.tile import TileContext  # noqa: E402

B, N, C, H = 4, 2048, 768, 12
D = C // H          # 64
HG = 2              # head groups (cores per batch)
HPC = H // HG       # heads per core = 6
CIN_T = C // 128    # 6 cin tiles
QT_TILES = 3        # 384 rows of Q^T (6 heads x 64) = 3 x 128
KT_TILES = N // 128  # 16
NCORES = 8
MASK_BIAS = -8192.0

F32 = mybir.dt.float32
F32R = mybir.dt.float32r
F16 = mybir.dt.float16
F8E5 = mybir.dt.float8e5

_CACHE = {}


def _V(k):
    return os.environ.get(k)


def _build_program():
    nc = bacc.Bacc(
        "TRN2",
        target_bir_lowering=False,
        debug=False,
        enable_asserts=False,
        num_devices=NCORES,
    )
    xT = nc.dram_tensor("xT", [C, N], F32R, kind="ExternalInput").ap()
    qkvT = nc.dram_tensor("qkvT", [C, 3 * HPC * D], F32R, kind="ExternalInput").ap()
    maskdr = nc.dram_tensor("maskdr", [N // 2, 2 * N], F8E5, kind="ExternalInput").ap()
    identdr = nc.dram_tensor("identdr", [64, 256], F8E5, kind="ExternalInput").ap()
    identdr2 = nc.dram_tensor("identdr2", [128, 256], F8E5, kind="ExternalInput").ap()
    projT = nc.dram_tensor("projT", [HPC * D, C], F16, kind="ExternalInput").ap()
    out = nc.dram_tensor("out", [N, C], F32, kind="ExternalOutput").ap()

    AL = mybir.AluOpType
    EXP = mybir.ActivationFunctionType.Exp
    LN = mybir.ActivationFunctionType.Ln

    with TileContext(nc) as tc:
        with tc.tile_pool(name="pers", bufs=1) as pers:
            QTs = [
                pers.tile([128, N], F32R, tag=f"qt{t}", name=f"qt{t}")
                for t in range(QT_TILES)
            ]
            KTs = [
                pers.tile([128, N], F32R, tag=f"kt{t}", name=f"kt{t}")
                for t in range(QT_TILES)
            ]
            Vaug = pers.tile([128, HPC * KT_TILES, D], F16, tag="vaug")
            Ocat = [
                pers.tile([128, N], F16, tag=f"oc{t}", name=f"oc{t}")
                for t in range(QT_TILES)
            ]
            PW = [
                pers.tile([128, C], F16, tag=f"pw{t}", name=f"pw{t}")
                for t in range(QT_TILES)
            ]
            if _V("KV_DR"):
                idr = pers.tile([128, 2, 128], F8E5, tag="idr")
                nc.sync.dma_start(
                    idr[:, :, :],
                    identdr2.rearrange("p (i q) -> p i q", i=2),
                )
            else:
                idr = pers.tile([128, 128], F8E5, tag="idr")
                nc.sync.dma_start(
                    idr[:, :], identdr.rearrange("p (i q) -> (p i) q", i=2)
                )
            for t in range(QT_TILES):
                nc.scalar.dma_start(PW[t][:, :], projT[t * 128 : (t + 1) * 128, :])

            # ================= Phase 1: QKV projection =================
            with (
                tc.tile_pool(name="ph1", bufs=1) as p1,
                tc.tile_pool(name="ph1p", bufs=4, space="PSUM") as p1p,
            ):
                xts = [
                    p1.tile([128, N], F32R, tag=f"x{ci}", name=f"x{ci}")
                    for ci in range(CIN_T)
                ]
                wts = [
                    p1.tile([128, 3 * HPC * D], F32R, tag=f"w{ci}", name=f"w{ci}")
                    for ci in range(CIN_T)
                ]
                for ci in range(CIN_T):
                    xeng = nc.sync if ci % 2 == 0 else nc.scalar
                    weng = nc.scalar if ci % 2 == 0 else nc.sync
                    xeng.dma_start(xts[ci][:, :], xT[ci * 128 : (ci + 1) * 128, :])
                    weng.dma_start(
                        wts[ci][:, :], qkvT[ci * 128 : (ci + 1) * 128, :]
                    )

                for which, dst in ((0, QTs), (1, KTs)):
                    off = which * HPC * D
                    for t in range(QT_TILES):
                        for qc in range(4):
                            ps = p1p.tile([128, 512], F32, tag="p1ps", name="ps")
                            for ci in range(CIN_T):
                                nc.tensor.matmul(
                                    ps[:, :],
                                    wts[ci][:, off + t * 128 : off + (t + 1) * 128],
                                    xts[ci][:, qc * 512 : (qc + 1) * 512],
                                    start=(ci == 0),
                                    stop=(ci == CIN_T - 1),
                                )
                            nc.vector.tensor_scalar(
                                dst[t][:, qc * 512 : (qc + 1) * 512],
                                ps[:, :], 0.0, None, op0=AL.add,
                            )

                voff = 2 * HPC * D
                for kt in range(KT_TILES):
                    ps = p1p.tile([128, HPC * D], F32, tag="p1ps", name="ps")
                    for ci in range(CIN_T):
                        nc.tensor.matmul(
                            ps[:, :],
                            xts[ci][:, kt * 128 : (kt + 1) * 128],
                            wts[ci][:, voff : voff + HPC * D],
                            start=(ci == 0),
                            stop=(ci == CIN_T - 1),
                        )
                    nc.scalar.copy(
                        Vaug[:, kt :: KT_TILES, :],
                        ps[:, :].rearrange("p (h d) -> p h d", h=HPC),
                    )

            # ================= Phase 2: attention =================
            with (
                tc.tile_pool(name="mk", bufs=2) as pmk,
                tc.tile_pool(name="work", bufs=2) as pw,
                tc.tile_pool(name="psS", bufs=3, space="PSUM") as psS,
                tc.tile_pool(name="psO", bufs=2, space="PSUM") as psO,
            ):
                pv_pending = []  # fine-grained PE filler closures

                def make_pv(hp, qc, PTs_blk):
                    """Two closure pairs per head: 8+8 PV matmuls, then post."""
                    parts = []
                    for a in range(2):
                        h = 2 * hp + a
                        ot = [None]

                        def chain_lo(a=a, h=h, PTs_blk=PTs_blk, ot=ot):
                            ot[0] = psO.tile([D, 512], F32, tag="ot", name="ot")
                            for kt in range(8):
                                nc.tensor.matmul(
                                    ot[0][:, :],
                                    Vaug[:, h * KT_TILES + kt, :],
                                    PTs_blk[a][0][:, kt, :],
                                    start=(kt == 0),
                                    stop=False,
                                )

                        def chain_hi(a=a, h=h, PTs_blk=PTs_blk, ot=ot,
                                     hp=hp, qc=qc):
                            for kt in range(8, KT_TILES):
                                nc.tensor.matmul(
                                    ot[0][:, :],
                                    Vaug[:, h * KT_TILES + kt, :],
                                    PTs_blk[a][1][:, kt - 8, :],
                                    start=False,
                                    stop=(kt == KT_TILES - 1),
                                )
                            nc.vector.tensor_scalar(
                                Ocat[hp][a * D : (a + 1) * D,
                                         qc * 512 : (qc + 1) * 512],
                                ot[0][:, :], 0.0, None, op0=AL.add,
                            )

                        parts.append(chain_lo)
                        parts.append(chain_hi)
                    return parts

                for qc in range(4):
                    mks = []
                    for j in range(4):
                        qt = qc * 4 + j
                        mk = pmk.tile([128, N], F8E5, tag=f"mk{j}", name=f"mk{j}")
                        nc.sync.dma_start(
                            mk[:, :],
                            maskdr[qt * 64 : (qt + 1) * 64, :].rearrange(
                                "p (i n) -> (p i) n", i=2
                            ),
                        )
                        mks.append(mk)

                    for hp in range(QT_TILES):
                        PTs = [
                            [
                                pw.tile(
                                    [128, 8, 512], F16, tag=f"pt{a}{hh}",
                                    name=f"PT{a}{hh}", bufs=2,
                                )
                                for hh in range(2)
                            ]
                            for a in range(2)
                        ]
                        for j in range(4):
                            qt = qc * 4 + j
                            for a in range(2):
                                mstat = pw.tile([128, 2], F32, tag=f"mstat{a}",
                                                name=f"mstat{a}", bufs=6)
                                negm = pw.tile([128, 1], F32, tag=f"negm{a}",
                                               name=f"negm{a}", bufs=6)
                                fs = pw.tile([128, 2], F32, tag=f"fs{a}",
                                             name=f"fs{a}", bufs=6)
                                lsum = pw.tile([128, 2], F32, tag=f"ls{a}",
                                               name=f"ls{a}", bufs=6)
                                lf = pw.tile([128, 2], F32, tag=f"lf{a}",
                                             name=f"lf{a}", bufs=6)
                                li = pw.tile([128, 1], F32, tag=f"li{a}",
                                             name=f"li{a}", bufs=6)
                                lr = pw.tile([128, 1], F32, tag=f"lr{a}",
                                             name=f"lr{a}", bufs=6)
                                fac = pw.tile([128, 2], F32, tag=f"fa{a}",
                                              name=f"fa{a}", bufs=6)
                                pnh = [
                                    pw.tile([128, 1024], F16, tag=f"pn{a}{hh}",
                                            name=f"pn{a}{hh}", bufs=2)
                                    for hh in range(2)
                                ]
                                for half in range(2):
                                    sp = psS.tile([128, 1024], F32, tag="sp",
                                                  name=f"sp{a}{half}")
                                    for c in range(2):
                                        kc = half * 1024 + c * 512
                                        nc.tensor.matmul(
                                            sp[:, c * 512 : (c + 1) * 512],
                                            QTs[hp][
                                                a * D : (a + 1) * D,
                                                qt * 128 : (qt + 1) * 128,
                                            ],
                                            KTs[hp][
                                                a * D : (a + 1) * D,
                                                kc : kc + 512,
                                            ],
                                            start=True,
                                            stop=False,
                                            tile_position=(a * D, 0),
                                        )
                                        nc.tensor.matmul(
                                            sp[:, c * 512 : (c + 1) * 512],
                                            idr[:, :],
                                            mks[j][:, kc : kc + 512],
                                            start=False,
                                            stop=True,
                                        )
                                    nc.vector.tensor_reduce(
                                        mstat[:, half : half + 1],
                                        sp[:, :],
                                        axis=mybir.AxisListType.X,
                                        op=AL.max,
                                        negate=True,
                                    )
                                    if _V("KV_LSUMDVE"):
                                        nc.scalar.activation(
                                            pnh[half][:, :],
                                            sp[:, :],
                                            EXP,
                                            bias=mstat[:, half : half + 1],
                                            scale=1.0,
                                        )
                                        nc.vector.tensor_reduce(
                                            lsum[:, half : half + 1],
                                            pnh[half][:, :],
                                            axis=mybir.AxisListType.X,
                                            op=AL.add,
                                        )
                                    else:
                                        nc.scalar.activation(
                                            pnh[half][:, :],
                                            sp[:, :],
                                            EXP,
                                            bias=mstat[:, half : half + 1],
                                            scale=1.0,
                                            accum_out=lsum[:, half : half + 1],
                                        )
                                nc.vector.tensor_reduce(
                                    negm[:, :],
                                    mstat[:, 0:2],
                                    axis=mybir.AxisListType.X,
                                    op=AL.min,
                                )
                                nc.scalar.activation(
                                    fs[:, :],
                                    mstat[:, 0:2],
                                    EXP,
                                    bias=negm[:, 0:1],
                                    scale=-1.0,
                                )
                                # l = lsum0*fs0 + lsum1*fs1; factor = fs/l
                                nc.vector.tensor_tensor(
                                    lf[:, :], lsum[:, :], fs[:, :], op=AL.mult
                                )
                                nc.vector.tensor_reduce(
                                    li[:, :], lf[:, :],
                                    axis=mybir.AxisListType.X, op=AL.add,
                                )
                                nc.vector.reciprocal(lr[:, :], li[:, :])
                                nc.vector.tensor_scalar(
                                    fac[:, :], fs[:, :], lr[:, 0:1], None,
                                    op0=AL.mult,
                                )
                                for half in range(2):
                                    nc.vector.tensor_scalar(
                                        pnh[half][:, :],
                                        pnh[half][:, :],
                                        fac[:, half : half + 1],
                                        None,
                                        op0=AL.mult,
                                    )
                                    nc.sync.dma_start_transpose(
                                        PTs[a][half][:, :, j * 128 : (j + 1) * 128],
                                        pnh[half][:, :],
                                    )
                                # PE filler between head groups
                                if not _V("KV_NOILV") and pv_pending:
                                    pv_pending.pop(0)()
                        pv_pending.extend(make_pv(hp, qc, PTs))
                        if _V("KV_NOILV") or hp == QT_TILES - 1:
                            while pv_pending:
                                pv_pending.pop(0)()

                    # proj for this q-chunk
                    for j in range(4):
                        qt = qc * 4 + j
                        y0 = psO.tile([128, 512], F32, tag="ot", name="y0")
                        y1 = psO.tile([128, 256], F32, tag="ot", name="y1")
                        for ct in range(QT_TILES):
                            lt = Ocat[ct][:, qt * 128 : (qt + 1) * 128]
                            nc.tensor.matmul(
                                y0[:, :], lt, PW[ct][:, 0:512],
                                start=(ct == 0), stop=(ct == QT_TILES - 1),
                            )
                            nc.tensor.matmul(
                                y1[:, :], lt, PW[ct][:, 512:768],
                                start=(ct == 0), stop=(ct == QT_TILES - 1),
                            )
                        ysb = pw.tile([128, C], F32, tag="ysb", name="ysb")
                        nc.vector.tensor_scalar(
                            ysb[:, 0:512], y0[:, :], 0.0, None, op0=AL.add
                        )
                        if _V("KV_Y1DVE"):
                            nc.vector.tensor_scalar(
                                ysb[:, 512:768], y1[:, :], 0.0, None, op0=AL.add
                            )
                        else:
                            nc.scalar.copy(ysb[:, 512:768], y1[:, :])
                        eng = nc.sync if qt % 2 == 0 else nc.scalar
                        eng.dma_start(
                            out[qt * 128 : (qt + 1) * 128, :], ysb[:, :]
                        )
    nc.compile()
    return nc


def kernel(x, local_attn_mask, qkv_w, proj_w, proj_b):
    x = np.asarray(x, dtype=np.float32)
    mask = np.asarray(local_attn_mask)
    qkv_w = np.asarray(qkv_w, dtype=np.float32)
    proj_w = np.asarray(proj_w, dtype=np.float32)
    proj_b = np.asarray(proj_b, dtype=np.float32)

    maskdr = (
        (float(MASK_BIAS) * mask.astype(np.float32))
        .reshape(N // 128, 64, 2, N)
        .reshape(N // 2, 2 * N)
        .astype(ml_dtypes.float8_e5m2)
    )
    identdr = (
        np.eye(128, dtype=np.float32)
        .reshape(64, 2, 128)
        .reshape(64, 256)
        .astype(ml_dtypes.float8_e5m2)
    )
    id2 = np.zeros((128, 2, 128), dtype=np.float32)
    id2[:, 0, :] = np.eye(128, dtype=np.float32)
    identdr2 = id2.reshape(128, 256).astype(ml_dtypes.float8_e5m2)
    in_maps = []
    for c in range(NCORES):
        b, hg = c // HG, c % HG
        rq = slice(hg * HPC * D, (hg + 1) * HPC * D)
        rk = slice(C + hg * HPC * D, C + (hg + 1) * HPC * D)
        rv = slice(2 * C + hg * HPC * D, 2 * C + (hg + 1) * HPC * D)
        wsel = np.concatenate(
            [qkv_w[rq] * float(D), qkv_w[rk], qkv_w[rv]], axis=0
        )
        in_maps.append(
            {
                "xT": np.ascontiguousarray(x[b].T),
                "qkvT": np.ascontiguousarray(wsel.T),
                "maskdr": maskdr,
                "identdr": identdr,
                "identdr2": identdr2,
                "projT": np.ascontiguousarray(
                    proj_w[:, hg * HPC * D : (hg + 1) * HPC * D].T
                ).astype(np.float16),
            }
        )

    if "nc" not in _CACHE:
        _CACHE["nc"] = _build_program()
    res = run_bass_kernel_spmd(_CACHE["nc"], in_maps, core_ids=list(range(NCORES)))
    _CACHE["last_results"] = res
    outs = res.results
    y = np.empty((B, N, C), dtype=np.float32)
    for b in range(B):
        y[b] = outs[2 * b]["out"] + outs[2 * b + 1]["out"] + proj_b[None, :]
    return y
